# revision 26
# baseline (speedup 1.0000x reference)
"""Fused SWA transformer layer (rmsnorm -> gated-conv MLP -> sliding-window attn)
for Trainium2, data-parallel over 8 NeuronCores with halo recompute.

v2: mixed-precision rewrite of the bf16 baseline.
  - MLP (up/gate/down) stays bf16 (fp8 too lossy there), block-pipelined
    through 512-col PSUM tiles so the PE never stalls on psum drains.
  - QKV + output projections run fp8e4 DoubleRow (K=256 per pass, 2x PE
    throughput). Weights are pre-scaled x32 host-side; activations x16;
    descale is folded into the psum->sbuf copies.
  - Scores use a compact bank-aligned [128, 768] psum layout per
    (chunk, head): s1|s2 full 256 queries, s0 only q<128, s3 only q>=128
    (the sliding window makes the other halves fully masked).
  - Softmax division: denominators from the ones-column of V, gathered
    per chunk into one [16, 256] tile, one batched reciprocal, then a
    PE broadcast and a single fused multiply per head.
"""
import sys
import numpy as np

sys.path.insert(0, "/opt/trn_rl_repo")
import ml_dtypes  # noqa: E402
import concourse.bass as bass  # noqa: E402
import concourse.tile as tile  # noqa: E402
from concourse import mybir  # noqa: E402

B, S, D = 2, 4096, 1024
DI = 2048
H, HD = 16, 64
WIN = 256
KC = 4
EPS = 1e-5
NCORES = 8
NEG = -30000.0   # additive mask for invalid keys
SHIFT = -8.0     # constant softmax shift folded into the mask
F32 = mybir.dt.float32
BF16 = mybir.dt.bfloat16
FP8 = mybir.dt.float8e4
AF = mybir.ActivationFunctionType
OP = mybir.AluOpType
DR = mybir.MatmulPerfMode.DoubleRow

# precision config
G3_FP8 = True    # qkv projections in fp8 DoubleRow
G4_FP8 = True    # output projection in fp8 DoubleRow
WSC = 32.0       # fp8 weight pre-scale
ASC = 16.0       # fp8 activation pre-scale


def _nslices(total, step):
    return [(i, min(step, total - i)) for i in range(0, total, step)]


def _view3(t, a, bc):
    """View a [128, a*bc] sbuf tile as a [128, a, bc] AP."""
    base = t[:]
    return bass.AP(t.tensor, base.offset, [base.ap[0], [bc, a], [1, bc]])


def _v3(t, W, kt, nk, col, n):
    """AP [128, nk, n] slice of a [128, KT*W] tile: dims (ktile, col)."""
    base = t[:]
    return bass.AP(t.tensor, base.offset + kt * W + col,
                   [base.ap[0], [W, nk], [1, n]])


def _split_excess_waits(nc, wsem, cap=1):
    """Walrus codegen caps embedded sync-wait commands per instruction. Move
    excess waits onto standalone EventSemaphore instructions inserted just
    before, on the same engine."""
    nid = 0
    for f in nc.m.functions:
        for blk in f.blocks:
            out = []
            changed = False
            for inst in blk.instructions:
                si = inst.sync_info
                if si is not None and len(si.on_wait) > cap:
                    waits = list(si.on_wait)
                    excess, keep = waits[:-cap], waits[-cap:]
                    while excess:
                        chunk, excess = excess[:2], excess[2:]
                        upd = mybir.SyncUpdate(
                            sync_type="semaphore", id=wsem.num,
                            ant_name=wsem.name, update_mode="sem-inc",
                            update_value=1)
                        es = mybir.InstEventSemaphore(
                            name=f"WSPLIT-{nid}", ins=[], outs=[],
                            engine=inst.engine,
                            sync_info=mybir.SyncInfo(
                                on_wait=chunk, on_update=[upd]))
                        nid += 1
                        out.append(es)
                    inst.sync_info = mybir.SyncInfo(
                        on_wait=keep, on_update=list(si.on_update))
                    changed = True
                out.append(inst)
            if changed:
                blk.instructions = out


def build_program(tmain, debug_taps=False):
    halo = WIN
    cext = tmain + halo          # x2/kv token range
    xext = cext + (KC - 1)       # u/z/x token range (conv needs 3 extra)
    nchunk = tmain // WIN
    KT = D // 128
    MT_DI = DI // 128
    MT_D = D // 128
    TT = cext // 128
    assert cext % 128 == 0 and tmain % WIN == 0

    qk_dt = FP8 if G3_FP8 else BF16
    o_dt = FP8 if G4_FP8 else BF16
    qkv_desc = 1.0 / (WSC * ASC) if G3_FP8 else 1.0
    o_desc = 1.0 / (WSC * ASC) if G4_FP8 else 1.0

    nc = bass.Bass("TRN2", target_bir_lowering=False, debug=False,
                   num_devices=NCORES)
    _wsem_cm = nc.semaphore()
    wsem = _wsem_cm.__enter__()
    nc._wsem_keepalive = _wsem_cm

    xT = nc.dram_tensor("xT", [D, xext], BF16, kind="ExternalInput")
    w_up = nc.dram_tensor("w_up", [DI, KT * 128], BF16, kind="ExternalInput")
    w_gate = nc.dram_tensor("w_gate", [DI, KT * 128], BF16,
                            kind="ExternalInput")
    w_down = nc.dram_tensor("w_down", [D, MT_DI * 128], BF16,
                            kind="ExternalInput")
    w_q = nc.dram_tensor("w_q", [D, KT * 128], qk_dt, kind="ExternalInput")
    w_k = nc.dram_tensor("w_k", [D, KT * 128], qk_dt, kind="ExternalInput")
    w_v = nc.dram_tensor("w_v", [128, KT * 1024], qk_dt, kind="ExternalInput")
    w_o = nc.dram_tensor("w_o", [D, KT * 128], o_dt, kind="ExternalInput")
    conv_w = nc.dram_tensor("conv_w", [DI, KC], F32, kind="ExternalInput")
    conv_b = nc.dram_tensor("conv_b", [DI], F32, kind="ExternalInput")
    mask0 = nc.dram_tensor("mask0", [128, 768], BF16, kind="ExternalInput")
    maskm = nc.dram_tensor("maskm", [128, 768], BF16, kind="ExternalInput")
    outT = nc.dram_tensor("outT", [D, tmain], F32, kind="ExternalOutput")
    taps = {}
    if debug_taps:
        for nm, shp, dt in (
                ("t_x2b0", [128, cext], BF16), ("t_h0", [128, cext], BF16),
                ("t_kt0", [128, cext], BF16), ("t_qt0", [128, tmain], BF16),
                ("t_vt0", [128, H * (HD + 1)], BF16),
                ("t_ee0", [128, 768], BF16), ("t_oc0", [65, 256], BF16),
                ("t_rowb0", [1, 16 * 256], BF16),
                ("t_aos", [128, (D // 128) * tmain],
                 FP8 if G4_FP8 else BF16)):
            taps[nm] = nc.dram_tensor(nm, shp, dt, kind="ExternalOutput")

    def tap(nm, ap):
        if debug_taps and nm in taps:
            t = taps.pop(nm)
            nc.sync.dma_start(t[:, :], ap)

    with tile.TileContext(nc) as tc:
        with tc.tile_pool(name="consts", bufs=1) as consts, \
             tc.tile_pool(name="x2b", bufs=1) as x2b_pool, \
             tc.tile_pool(name="x28p", bufs=1) as x28_pool, \
             tc.tile_pool(name="aos", bufs=1) as aos_pool:
            # ---- constants ----
            m0_sb = consts.tile([128, 768], BF16)
            mm_sb = consts.tile([128, 768], BF16)
            nc.sync.dma_start(m0_sb[:], mask0[:, :])
            nc.sync.dma_start(mm_sb[:], maskm[:, :])
            cw_sb = consts.tile([128, MT_DI * KC], F32)
            nc.sync.dma_start(
                _view3(cw_sb, MT_DI, KC),
                bass.AP(conv_w, 0, [[KC, 128], [128 * KC, MT_DI], [1, KC]]))
            cb_sb = consts.tile([128, MT_DI], F32)
            nc.sync.dma_start(
                cb_sb[:], bass.AP(conv_b, 0, [[1, 128], [128, MT_DI]]))
            eps_sb = consts.tile([128, 1], F32)
            nc.vector.memset(eps_sb[:], EPS)
            ones_sb = consts.tile([128, 1], BF16)
            nc.vector.memset(ones_sb[:], 1.0)
            ones_rowf = consts.tile([1, 128], F32)
            nc.vector.memset(ones_rowf[:], 1.0)
            ones_rowb = consts.tile([1, 128], BF16)
            nc.vector.memset(ones_rowb[:], ASC if G4_FP8 else 1.0)

            x2b = [x2b_pool.tile([128, cext], BF16, name=f"x2b{i}")
                   for i in range(MT_D)]
            x28 = (x28_pool.tile([128, KT * cext], FP8, name="x28")
                   if G3_FP8 else None)
            # aos: fp8 k-major [128, KT*tmain] or bf16 per-hp tiles
            if G4_FP8:
                aos8 = aos_pool.tile([128, KT * tmain], FP8, name="aos8")
            else:
                aosb = [aos_pool.tile([128, tmain], BF16, name=f"ao{i}")
                        for i in range(MT_D)]

            with tc.tile_pool(name="xT", bufs=KT) as xT_pool, \
                 tc.tile_pool(name="zT", bufs=KT) as zT_pool, \
                 tc.tile_pool(name="h", bufs=MT_DI) as h_pool:
                xts = []
                for k in range(KT):
                    xt = xT_pool.tile([128, xext], BF16)
                    nc.sync.dma_start(xt[:], xT[k * 128:(k + 1) * 128, :])
                    xts.append(xt)

                # ---- rmsnorm scale r = 1/sqrt(mean(x^2)+eps) ----
                zts = []
                with tc.tile_pool(name="ss_ps", bufs=1, space="PSUM") as ssp, \
                     tc.tile_pool(name="sq", bufs=3) as sq_pool:
                    ss_ps = ssp.tile([1, xext], F32)
                    for k in range(KT):
                        sq = sq_pool.tile([128, xext], BF16)
                        nc.vector.tensor_mul(sq[:], xts[k][:], xts[k][:])
                        for (o, n) in _nslices(xext, 512):
                            nc.tensor.matmul(
                                ss_ps[:, o:o + n], ones_sb[:], sq[:, o:o + n],
                                start=(k == 0), stop=(k == KT - 1))
                    t_sb = sq_pool.tile([1, xext], F32)
                    nc.scalar.activation(t_sb[:], ss_ps[:], AF.Sqrt,
                                         bias=eps_sb[0:1, :], scale=1.0 / D)
                    r_sb = sq_pool.tile([1, xext], F32)
                    nc.vector.reciprocal(r_sb[:], t_sb[:])
                    rb_ps = ssp.tile([128, xext], F32)
                    for (o, n) in _nslices(xext, 512):
                        nc.tensor.matmul(
                            rb_ps[:, o:o + n], ones_rowf[:], r_sb[:, o:o + n],
                            start=True, stop=True)
                    for k in range(KT):
                        zt = zT_pool.tile([128, xext], BF16)
                        nc.vector.tensor_tensor(
                            out=zt[:], in0=xts[k][:], in1=rb_ps[:],
                            op=OP.mult)
                        zts.append(zt)

                # ---- up/gate projections + causal dwconv + silu -> h ----
                hs = []
                with tc.tile_pool(name="wA", bufs=3) as wA_pool, \
                     tc.tile_pool(name="upps", bufs=3, space="PSUM") as upp, \
                     tc.tile_pool(name="gaps", bufs=3, space="PSUM") as gap, \
                     tc.tile_pool(name="usb", bufs=2) as u_pool, \
                     tc.tile_pool(name="silp", bufs=2) as sil_pool, \
                     tc.tile_pool(name="convt", bufs=2) as cv_pool:
                    for m in range(MT_DI):
                        wu = wA_pool.tile([128, KT * 128], BF16, tag="wu")
                        nc.sync.dma_start(
                            wu[:], w_up[m * 128:(m + 1) * 128, :])
                        wg = wA_pool.tile([128, KT * 128], BF16, tag="wg")
                        nc.sync.dma_start(
                            wg[:], w_gate[m * 128:(m + 1) * 128, :])

                        u_sb = u_pool.tile([128, xext], BF16)
                        for (o, n) in _nslices(xext, 512):
                            up_b = upp.tile([128, 512], F32)
                            for k in range(KT):
                                nc.tensor.matmul(
                                    up_b[:, 0:n],
                                    wu[:, k * 128:(k + 1) * 128],
                                    zts[k][:, o:o + n],
                                    start=(k == 0), stop=(k == KT - 1))
                            nc.scalar.activation(
                                u_sb[:, o:o + n], up_b[:, 0:n], AF.Copy)
                        sil = sil_pool.tile([128, cext], BF16)
                        for (o, n) in _nslices(cext, 512):
                            ga_b = gap.tile([128, 512], F32)
                            for k in range(KT):
                                nc.tensor.matmul(
                                    ga_b[:, 0:n],
                                    wg[:, k * 128:(k + 1) * 128],
                                    zts[k][:, KC - 1 + o:KC - 1 + o + n],
                                    start=(k == 0), stop=(k == KT - 1))
                            nc.scalar.activation(
                                sil[:, o:o + n], ga_b[:, 0:n], AF.Silu)
                        # dwconv: acc = sum_j u[:, j:j+cext]*cw[:,j] + b
                        acc = cv_pool.tile([128, cext], BF16, tag="acc")
                        nc.vector.tensor_scalar(
                            out=acc[:], in0=u_sb[:, 0:cext],
                            scalar1=cw_sb[:, m * KC:m * KC + 1],
                            scalar2=cb_sb[:, m:m + 1],
                            op0=OP.mult, op1=OP.add)
                        for j in range(1, KC):
                            tmp = cv_pool.tile([128, cext], BF16, tag="tmp")
                            nc.vector.tensor_scalar(
                                out=tmp[:], in0=u_sb[:, j:j + cext],
                                scalar1=cw_sb[:, m * KC + j:m * KC + j + 1],
                                scalar2=None, op0=OP.mult)
                            nc.vector.tensor_add(acc[:], acc[:], tmp[:])
                        h = h_pool.tile([128, cext], BF16)
                        nc.vector.tensor_mul(h[:], sil[:], acc[:])
                        hs.append(h)

                # ---- down projection + residual -> x2 ----
                with tc.tile_pool(name="wD", bufs=2) as wD_pool, \
                     tc.tile_pool(name="dps", bufs=3, space="PSUM") as d_pool:
                    for m in range(MT_D):
                        wd = wD_pool.tile([128, MT_DI * 128], BF16)
                        nc.sync.dma_start(
                            wd[:], w_down[m * 128:(m + 1) * 128, :])
                        for (o, n) in _nslices(cext, 512):
                            d_b = d_pool.tile([128, 512], F32)
                            for k in range(MT_DI):
                                nc.tensor.matmul(
                                    d_b[:, 0:n],
                                    wd[:, k * 128:(k + 1) * 128],
                                    hs[k][:, o:o + n],
                                    start=(k == 0), stop=(k == MT_DI - 1))
                            nc.vector.tensor_tensor(
                                out=x2b[m][:, o:o + n], in0=d_b[:, 0:n],
                                in1=xts[m][:, KC - 1 + o:KC - 1 + o + n],
                                op=OP.add)
                            if G3_FP8:
                                nc.scalar.activation(
                                    out=_v3(x28, cext, m, 1, o, n),
                                    in_=x2b[m][:, o:o + n],
                                    func=AF.Copy, scale=ASC)
                        if m == 0:
                            tap("t_x2b0", x2b[0][:])
                            tap("t_h0", hs[0][:])

            # ---- qkv projections ----
            with tc.tile_pool(name="qT", bufs=MT_D) as q_pool, \
                 tc.tile_pool(name="kT", bufs=MT_D) as k_pool, \
                 tc.tile_pool(name="vtm", bufs=TT) as v_pool, \
                 tc.tile_pool(name="wOr", bufs=1) as wO_pool, \
                 tc.tile_pool(name="osb", bufs=3) as out_pool:
                kts = []
                qts = []
                with tc.tile_pool(name="wK", bufs=3) as wK_pool, \
                     tc.tile_pool(name="kps", bufs=3, space="PSUM") as kpp:
                    for m in range(MT_D):
                        wk = wK_pool.tile([128, KT * 128], qk_dt, tag="wk")
                        nc.sync.dma_start(wk[:], w_k[m * 128:(m + 1) * 128, :])
                        kt = k_pool.tile([128, cext], BF16)
                        for (o, n) in _nslices(cext, 512):
                            k_b = kpp.tile([128, 512], F32, tag="kb")
                            if G3_FP8:
                                for k in range(0, KT, 2):
                                    nc.tensor.matmul(
                                        k_b[:, 0:n],
                                        _v3(wk, 128, k, 2, 0, 128),
                                        _v3(x28, cext, k, 2, o, n),
                                        start=(k == 0), stop=(k == KT - 2),
                                        perf_mode=DR)
                            else:
                                for k in range(KT):
                                    nc.tensor.matmul(
                                        k_b[:, 0:n],
                                        wk[:, k * 128:(k + 1) * 128],
                                        x2b[k][:, o:o + n],
                                        start=(k == 0), stop=(k == KT - 1))
                            nc.scalar.activation(
                                kt[:, o:o + n], k_b[:, 0:n], AF.Copy,
                                scale=qkv_desc)
                        kts.append(kt)
                        if m == 0:
                            tap("t_kt0", kt[:])
                    for m in range(MT_D):
                        wq = wK_pool.tile([128, KT * 128], qk_dt, tag="wq")
                        nc.sync.dma_start(wq[:], w_q[m * 128:(m + 1) * 128, :])
                        qt = q_pool.tile([128, tmain], BF16)
                        for (o, n) in _nslices(tmain, 512):
                            q_b = kpp.tile([128, 512], F32, tag="qb")
                            if G3_FP8:
                                for k in range(0, KT, 2):
                                    nc.tensor.matmul(
                                        q_b[:, 0:n],
                                        _v3(wq, 128, k, 2, 0, 128),
                                        _v3(x28, cext, k, 2, halo + o, n),
                                        start=(k == 0), stop=(k == KT - 2),
                                        perf_mode=DR)
                            else:
                                for k in range(KT):
                                    nc.tensor.matmul(
                                        q_b[:, 0:n],
                                        wq[:, k * 128:(k + 1) * 128],
                                        x2b[k][:, halo + o:halo + o + n],
                                        start=(k == 0), stop=(k == KT - 1))
                            nc.scalar.activation(
                                qt[:, o:o + n], q_b[:, 0:n], AF.Copy,
                                scale=qkv_desc)
                        qts.append(qt)
                        if m == 0:
                            tap("t_qt0", qt[:])

                wos = []
                for m in range(MT_D):
                    wo = wO_pool.tile([128, KT * 128], o_dt, name=f"wo{m}")
                    nc.sync.dma_start(wo[:], w_o[m * 128:(m + 1) * 128, :])
                    wos.append(wo)
                vts = []
                with tc.tile_pool(name="wV", bufs=1) as wV_pool, \
                     tc.tile_pool(name="vps", bufs=2, space="PSUM") as vpp:
                    wv = wV_pool.tile([128, KT * 1024], qk_dt)
                    nc.sync.dma_start(wv[:], w_v[:, :])
                    for tt in range(TT):
                        v_ps = vpp.tile([128, 1024], F32)
                        if G3_FP8:
                            for (o, n) in _nslices(1024, 512):
                                for k in range(0, KT, 2):
                                    nc.tensor.matmul(
                                        v_ps[:, o:o + n],
                                        _v3(x28, cext, k, 2, tt * 128, 128),
                                        _v3(wv, 1024, k, 2, o, n),
                                        start=(k == 0), stop=(k == KT - 2),
                                        perf_mode=DR)
                        else:
                            for (o, n) in _nslices(1024, 512):
                                for k in range(KT):
                                    nc.tensor.matmul(
                                        v_ps[:, o:o + n],
                                        x2b[k][:, tt * 128:(tt + 1) * 128],
                                        wv[:, k * 1024 + o:k * 1024 + o + n],
                                        start=(k == 0), stop=(k == KT - 1))
                        vt = v_pool.tile([128, H * (HD + 1)], BF16)
                        nc.vector.memset(
                            bass.AP(vt.tensor, vt[:].offset + HD,
                                    [vt[:].ap[0], [HD + 1, H], [1, 1]]), 1.0)
                        for (o, n) in _nslices(1024, 512):
                            nh = n // HD
                            dst = bass.AP(vt.tensor,
                                          vt[:].offset + (o // HD) * (HD + 1),
                                          [vt[:].ap[0], [HD + 1, nh], [1, HD]])
                            src = bass.AP(v_ps.tensor, v_ps[:].offset + o,
                                          [v_ps[:].ap[0], [HD, nh], [1, HD]])
                            nc.scalar.activation(dst, src, AF.Copy,
                                                 scale=qkv_desc)
                        vts.append(vt)
                        if tt == 0:
                            tap("t_vt0", vt[:])

                # ---- sliding-window attention ----
                # compact scores layout: cols [0:256]=s1(q 0:256),
                # [256:512]=s2(q 0:256), [512:640]=s0(q 0:128),
                # [640:768]=s3(q 128:256)
                with tc.tile_pool(name="sps", bufs=2, space="PSUM") as s_pool, \
                     tc.tile_pool(name="ops", bufs=2, space="PSUM") as o_pool, \
                     tc.tile_pool(name="rbps", bufs=2, space="PSUM") as rb_pool, \
                     tc.tile_pool(name="adp", bufs=3) as ad_pool, \
                     tc.tile_pool(name="esb", bufs=3) as e_pool, \
                     tc.tile_pool(name="ocp", bufs=36) as oc_pool, \
                     tc.tile_pool(name="denp", bufs=2) as den_pool, \
                     tc.tile_pool(name="rcpp", bufs=2) as rcp_pool:
                    pend = []

                    def scores_stage(c, hh):
                        msk = m0_sb if c == 0 else mm_sb
                        hp, x = hh >> 1, hh & 1
                        kt, qt = kts[hp], qts[hp]
                        po = x * 64
                        s_ps = s_pool.tile([128, 768], F32, tag="spt")
                        qb = c * WIN
                        for (dst, ks, qo, qn) in (
                                (0, 1, 0, 256), (256, 2, 0, 256),
                                (512, 0, 0, 128), (640, 3, 128, 128)):
                            nc.tensor.matmul(
                                s_ps[:, dst:dst + qn],
                                kt[po:po + 64,
                                   qb + ks * 128:qb + (ks + 1) * 128],
                                qt[po:po + 64, qb + qo:qb + qo + qn],
                                start=True, stop=True,
                                tile_position=(po, 0))
                        ad = ad_pool.tile([128, 768], BF16)
                        nc.vector.tensor_tensor(
                            out=ad[:], in0=s_ps[:], in1=msk[:], op=OP.add)
                        ee = e_pool.tile([128, 768], BF16)
                        nc.scalar.activation(ee[:], ad[:], AF.Exp)
                        return ee

                    def av_stage(c, hh, ee, den_sb, ocs):
                        o_ps = o_pool.tile([65, 256], F32)
                        vsl = [vts[c * 2 + s][
                            :, hh * (HD + 1):(hh + 1) * (HD + 1)]
                            for s in range(4)]
                        nc.tensor.matmul(o_ps[:, 0:256], vsl[1],
                                         ee[:, 0:256],
                                         start=True, stop=False)
                        nc.tensor.matmul(o_ps[:, 0:256], vsl[2],
                                         ee[:, 256:512],
                                         start=False, stop=False,
                                         skip_group_check=True)
                        nc.tensor.matmul(o_ps[:, 0:128], vsl[0],
                                         ee[:, 512:640],
                                         start=False, stop=False,
                                         skip_group_check=True)
                        nc.tensor.matmul(o_ps[:, 128:256], vsl[3],
                                         ee[:, 640:768],
                                         start=False, stop=True,
                                         skip_group_check=True)
                        oc = oc_pool.tile([65, 256], BF16)
                        nc.scalar.activation(oc[:], o_ps[:, :], AF.Copy)
                        ocs.append(oc)
                        nc.sync.dma_start(den_sb[hh:hh + 1, :],
                                          oc[64:65, :])

                    def division_start(c, den_sb):
                        rcp = rcp_pool.tile([16, 256], BF16, tag="rc")
                        with nc.allow_low_precision(
                                reason="softmax denom recip in bf16"):
                            nc.vector.reciprocal(rcp[:], den_sb[:])
                        rowb = rcp_pool.tile([1, 16 * 256], BF16, tag="rw")
                        nc.sync.dma_start(
                            bass.AP(rowb.tensor, rowb[:].offset,
                                    [[rowb[:].ap[0][0], 1], [256, 16],
                                     [1, 256]]),
                            rcp[:])
                        return rowb

                    def division_head(c, hh, rowb, ocs):
                        hp, x = hh >> 1, hh & 1
                        rb2 = rb_pool.tile([64, 256], F32)
                        nc.tensor.matmul(
                            rb2[:], ones_rowb[:, 0:64],
                            rowb[0:1, hh * 256:(hh + 1) * 256],
                            start=True, stop=True)
                        if G4_FP8:
                            dst = aos8[x * 64:(x + 1) * 64,
                                       hp * tmain + c * WIN:
                                       hp * tmain + (c + 1) * WIN]
                        else:
                            dst = aosb[hp][x * 64:(x + 1) * 64,
                                           c * WIN:(c + 1) * WIN]
                        nc.vector.tensor_tensor(
                            out=dst, in0=ocs[hh][0:64, :],
                            in1=rb2[:], op=OP.mult)

                    def oproj_chunk(c):
                        ob = c * WIN
                        for m in range(MT_D):
                            wo_b = s_pool.tile([128, 768], F32, tag="spt")
                            if G4_FP8:
                                for k in range(0, KT, 2):
                                    nc.tensor.matmul(
                                        wo_b[:, 0:WIN],
                                        _v3(wos[m], 128, k, 2, 0, 128),
                                        _v3(aos8, tmain, k, 2, ob, WIN),
                                        start=(k == 0), stop=(k == KT - 2),
                                        perf_mode=DR)
                            else:
                                for k in range(KT):
                                    nc.tensor.matmul(
                                        wo_b[:, 0:WIN],
                                        wos[m][:, k * 128:(k + 1) * 128],
                                        aosb[k][:, ob:ob + WIN],
                                        start=(k == 0), stop=(k == KT - 1))
                            ot = out_pool.tile([128, 256], F32)
                            nc.vector.scalar_tensor_tensor(
                                out=ot[:], in0=wo_b[:, 0:WIN], scalar=o_desc,
                                in1=x2b[m][:, halo + ob:halo + ob + WIN],
                                op0=OP.mult, op1=OP.add)
                            nc.sync.dma_start(
                                outT[m * 128:(m + 1) * 128, ob:ob + WIN],
                                ot[:])

                    dens, ocss = {}, {}
                    divq = []      # chunks whose AV is fully emitted
                    divheads = []  # (c, hh, rowb) division pairs to spread
                    oprojq = []    # chunks whose division is fully emitted

                    def pop_av():
                        pc, ph, pee = pend.pop(0)
                        av_stage(pc, ph, pee, dens[pc], ocss[pc])
                        if ph == H - 1:
                            divq.append(pc)

                    def pump(ndiv):
                        if divq and not divheads:
                            dc = divq.pop(0)
                            rw = division_start(dc, dens[dc])
                            divheads.extend(
                                (dc, hh, rw) for hh in range(H))
                        for _ in range(ndiv):
                            if divheads:
                                dc, dh, rw = divheads.pop(0)
                                division_head(dc, dh, rw, ocss[dc])
                                if dh == H - 1:
                                    oprojq.append(dc)

                    for c in range(nchunk):
                        dens[c] = den_pool.tile([16, 256], BF16,
                                                name=f"den{c & 1}")
                        ocss[c] = []
                        for hh in range(H):
                            pend.append((c, hh, scores_stage(c, hh)))
                            if len(pend) > 1:
                                pop_av()
                            if hh >= 4:
                                pump(2)
                            if oprojq and hh == 14:
                                oproj_chunk(oprojq.pop(0))
                    while pend:
                        pop_av()
                    while divheads or divq:
                        pump(4)
                    while oprojq:
                        oproj_chunk(oprojq.pop(0))

    _split_excess_waits(nc, wsem)
    return nc


def make_masks():
    """Compact [128, 768] masks: cols [0:256]=s1, [256:512]=s2,
    [512:640]=s0 (q 0:128), [640:768]=s3 (q 128:256)."""
    i = np.arange(WIN)[None, :]          # query
    p = np.arange(2 * WIN)[:, None]      # key
    band = (p > i) & (p <= i + WIN)
    mm_full = np.where(band, SHIFT, NEG).astype(np.float32)   # [512, 256]
    m0_full = np.where(band & (p >= WIN), SHIFT, NEG).astype(np.float32)

    def compact(mfull):
        out = np.full((128, 768), NEG, np.float32)
        out[:, 0:256] = mfull[128:256, :]          # s1
        out[:, 256:512] = mfull[256:384, :]        # s2
        out[:, 512:640] = mfull[0:128, 0:128]      # s0, q<128
        out[:, 640:768] = mfull[384:512, 128:256]  # s3, q>=128
        return out
    return compact(m0_full), compact(mm_full)


def _lhsT_img(W, scale=1.0):
    """[Din, Dout] weight -> lhsT image [Dout, KTin*128] with
    img[mo*128+f, k*128+p] = W[k*128+p, mo*128+f]*scale."""
    Din, Dout = W.shape
    KTin = Din // 128
    MT = Dout // 128
    img = (W * scale).reshape(KTin, 128, MT, 128).transpose(2, 1, 0, 3)
    return np.ascontiguousarray(img.reshape(Dout, KTin * 128))


def make_core_inputs(x, norm_w, w_up, w_gate, w_down, conv_w, conv_b,
                     w_qkv, w_o, tmain, s_total):
    halo = WIN
    xext = tmain + halo + (KC - 1)
    bf = ml_dtypes.bfloat16
    f8 = mybir.dt.np(FP8)
    qk_np = f8 if G3_FP8 else bf
    o_np = f8 if G4_FP8 else bf
    qk_sc = WSC if G3_FP8 else 1.0
    o_sc = WSC if G4_FP8 else 1.0

    wu_img = _lhsT_img(norm_w[:, None] * w_up).astype(bf)
    wg_img = _lhsT_img(norm_w[:, None] * w_gate).astype(bf)
    wd_img = _lhsT_img(w_down).astype(bf)
    wq = w_qkv[:, 0:D] * np.float32(1.0 / np.sqrt(HD))
    wq_img = _lhsT_img(wq, qk_sc).astype(qk_np)
    wk_img = _lhsT_img(w_qkv[:, D:2 * D], qk_sc).astype(qk_np)
    # v weights as rhs image: [128, KT*1024], [p, k*1024+f]
    wv = (w_qkv[:, 2 * D:3 * D] * qk_sc).reshape(D // 128, 128, D)
    wv_img = np.ascontiguousarray(
        wv.transpose(1, 0, 2).reshape(128, (D // 128) * D)).astype(qk_np)
    wo_img = _lhsT_img(w_o, o_sc).astype(o_np)
    m0, mm = make_masks()
    m0 = m0.astype(bf)
    mm = mm.astype(bf)
    cores_per_b = s_total // tmain
    in_maps = []
    for c in range(NCORES):
        b, blk = divmod(c, cores_per_b)
        t0 = blk * tmain
        lo = t0 - halo - (KC - 1)
        xe = x[b, max(0, lo):t0 + tmain]
        if lo < 0:
            xe = np.concatenate(
                [np.zeros((-lo, D), np.float32), xe], axis=0)
        xT = np.ascontiguousarray(xe.T).astype(bf)
        assert xT.shape == (D, xext)
        in_maps.append({
            "xT": xT,
            "w_up": wu_img, "w_gate": wg_img, "w_down": wd_img,
            "w_q": wq_img, "w_k": wk_img, "w_v": wv_img, "w_o": wo_img,
            "conv_w": conv_w.astype(np.float32),
            "conv_b": conv_b.astype(np.float32),
            "mask0": m0 if blk == 0 else mm,
            "maskm": mm,
        })
    return in_maps


_CACHED = {}


def kernel(x, norm_w, w_up, w_gate, w_down, conv_w, conv_b, w_qkv, w_o):
    from concourse.bass_utils import run_bass_kernel_spmd
    tmain = (B * S) // NCORES
    if "nc" not in _CACHED:
        _CACHED["nc"] = build_program(tmain)
    nc = _CACHED["nc"]
    in_maps = make_core_inputs(
        np.asarray(x, np.float32), np.asarray(norm_w, np.float32),
        np.asarray(w_up, np.float32), np.asarray(w_gate, np.float32),
        np.asarray(w_down, np.float32), np.asarray(conv_w, np.float32),
        np.asarray(conv_b, np.float32), np.asarray(w_qkv, np.float32),
        np.asarray(w_o, np.float32), tmain, S)
    res = run_bass_kernel_spmd(nc, in_maps, core_ids=list(range(NCORES)))
    out = np.empty((B, S, D), np.float32)
    cores_per_b = S // tmain
    for c in range(NCORES):
        b, blk = divmod(c, cores_per_b)
        out[b, blk * tmain:(blk + 1) * tmain] = res.results[c]["outT"].T
    return out


# revision 29
# speedup vs baseline: 1.0476x; 1.0476x over previous
"""Fused SWA transformer layer (rmsnorm -> gated-conv MLP -> sliding-window attn)
for Trainium2, data-parallel over 8 NeuronCores with halo recompute.

v2: mixed-precision rewrite of the bf16 baseline.
  - MLP (up/gate/down) stays bf16 (fp8 too lossy there), block-pipelined
    through 512-col PSUM tiles so the PE never stalls on psum drains.
  - QKV + output projections run fp8e4 DoubleRow (K=256 per pass, 2x PE
    throughput). Weights are pre-scaled x32 host-side; activations x16;
    descale is folded into the psum->sbuf copies.
  - Scores use a compact bank-aligned [128, 768] psum layout per
    (chunk, head): s1|s2 full 256 queries, s0 only q<128, s3 only q>=128
    (the sliding window makes the other halves fully masked).
  - Softmax division: denominators from the ones-column of V, gathered
    per chunk into one [16, 256] tile, one batched reciprocal, then a
    PE broadcast and a single fused multiply per head.
"""
import sys
import numpy as np

sys.path.insert(0, "/opt/trn_rl_repo")
import ml_dtypes  # noqa: E402
import concourse.bass as bass  # noqa: E402
import concourse.tile as tile  # noqa: E402
from concourse import mybir  # noqa: E402

B, S, D = 2, 4096, 1024
DI = 2048
H, HD = 16, 64
WIN = 256
KC = 4
EPS = 1e-5
NCORES = 8
NEG = -30000.0   # additive mask for invalid keys
SHIFT = -8.0     # constant softmax shift folded into the mask
F32 = mybir.dt.float32
BF16 = mybir.dt.bfloat16
FP8 = mybir.dt.float8e4
AF = mybir.ActivationFunctionType
OP = mybir.AluOpType
DR = mybir.MatmulPerfMode.DoubleRow

# precision config
G3_FP8 = True    # qkv projections in fp8 DoubleRow
G4_FP8 = True    # output projection in fp8 DoubleRow
WSC = 32.0       # fp8 weight pre-scale
ASC = 16.0       # fp8 activation pre-scale


def _nslices(total, step):
    return [(i, min(step, total - i)) for i in range(0, total, step)]


def _view3(t, a, bc):
    """View a [128, a*bc] sbuf tile as a [128, a, bc] AP."""
    base = t[:]
    return bass.AP(t.tensor, base.offset, [base.ap[0], [bc, a], [1, bc]])


def _v3(t, W, kt, nk, col, n):
    """AP [128, nk, n] slice of a [128, KT*W] tile: dims (ktile, col)."""
    base = t[:]
    return bass.AP(t.tensor, base.offset + kt * W + col,
                   [base.ap[0], [W, nk], [1, n]])


def _split_excess_waits(nc, wsem, cap=1):
    """Walrus codegen caps embedded sync-wait commands per instruction. Move
    excess waits onto standalone EventSemaphore instructions inserted just
    before, on the same engine."""
    nid = 0
    for f in nc.m.functions:
        for blk in f.blocks:
            out = []
            changed = False
            for inst in blk.instructions:
                si = inst.sync_info
                if si is not None and len(si.on_wait) > cap:
                    waits = list(si.on_wait)
                    excess, keep = waits[:-cap], waits[-cap:]
                    while excess:
                        chunk, excess = excess[:2], excess[2:]
                        upd = mybir.SyncUpdate(
                            sync_type="semaphore", id=wsem.num,
                            ant_name=wsem.name, update_mode="sem-inc",
                            update_value=1)
                        es = mybir.InstEventSemaphore(
                            name=f"WSPLIT-{nid}", ins=[], outs=[],
                            engine=inst.engine,
                            sync_info=mybir.SyncInfo(
                                on_wait=chunk, on_update=[upd]))
                        nid += 1
                        out.append(es)
                    inst.sync_info = mybir.SyncInfo(
                        on_wait=keep, on_update=list(si.on_update))
                    changed = True
                out.append(inst)
            if changed:
                blk.instructions = out


def build_program(tmain, debug_taps=False):
    halo = WIN
    cext = tmain + halo          # x2/kv token range
    xext = cext + (KC - 1)       # u/z/x token range (conv needs 3 extra)
    nchunk = tmain // WIN
    KT = D // 128
    MT_DI = DI // 128
    MT_D = D // 128
    TT = cext // 128
    assert cext % 128 == 0 and tmain % WIN == 0

    qk_dt = FP8 if G3_FP8 else BF16
    o_dt = FP8 if G4_FP8 else BF16
    qkv_desc = 1.0 / (WSC * ASC) if G3_FP8 else 1.0
    o_desc = 1.0 / (WSC * ASC) if G4_FP8 else 1.0

    nc = bass.Bass("TRN2", target_bir_lowering=False, debug=False,
                   num_devices=NCORES)
    _wsem_cm = nc.semaphore()
    wsem = _wsem_cm.__enter__()
    nc._wsem_keepalive = _wsem_cm

    xT = nc.dram_tensor("xT", [D, xext], BF16, kind="ExternalInput")
    w_up = nc.dram_tensor("w_up", [DI, KT * 128], BF16, kind="ExternalInput")
    w_gate = nc.dram_tensor("w_gate", [DI, KT * 128], BF16,
                            kind="ExternalInput")
    w_down = nc.dram_tensor("w_down", [D, MT_DI * 128], BF16,
                            kind="ExternalInput")
    w_q = nc.dram_tensor("w_q", [D, KT * 128], qk_dt, kind="ExternalInput")
    w_k = nc.dram_tensor("w_k", [D, KT * 128], qk_dt, kind="ExternalInput")
    w_v = nc.dram_tensor("w_v", [128, KT * 1024], qk_dt, kind="ExternalInput")
    w_o = nc.dram_tensor("w_o", [D, KT * 128], o_dt, kind="ExternalInput")
    conv_w = nc.dram_tensor("conv_w", [DI, KC], F32, kind="ExternalInput")
    conv_b = nc.dram_tensor("conv_b", [DI], F32, kind="ExternalInput")
    mask0 = nc.dram_tensor("mask0", [128, 768], BF16, kind="ExternalInput")
    maskm = nc.dram_tensor("maskm", [128, 768], BF16, kind="ExternalInput")
    outT = nc.dram_tensor("outT", [D, tmain], F32, kind="ExternalOutput")
    taps = {}
    if debug_taps:
        for nm, shp, dt in (
                ("t_x2b0", [128, cext], BF16), ("t_h0", [128, cext], BF16),
                ("t_kt0", [128, cext], BF16), ("t_qt0", [128, tmain], BF16),
                ("t_vt0", [128, H * (HD + 1)], BF16),
                ("t_ee0", [128, 768], BF16), ("t_oc0", [65, 256], BF16),
                ("t_rowb0", [1, 16 * 256], BF16),
                ("t_aos", [128, (D // 128) * tmain],
                 FP8 if G4_FP8 else BF16)):
            taps[nm] = nc.dram_tensor(nm, shp, dt, kind="ExternalOutput")

    def tap(nm, ap):
        if debug_taps and nm in taps:
            t = taps.pop(nm)
            nc.sync.dma_start(t[:, :], ap)

    with tile.TileContext(nc) as tc:
        with tc.tile_pool(name="consts", bufs=1) as consts, \
             tc.tile_pool(name="x2b", bufs=1) as x2b_pool, \
             tc.tile_pool(name="x28p", bufs=1) as x28_pool, \
             tc.tile_pool(name="aos", bufs=1) as aos_pool:
            # ---- constants ----
            m0_sb = consts.tile([128, 768], BF16)
            mm_sb = consts.tile([128, 768], BF16)
            nc.sync.dma_start(m0_sb[:], mask0[:, :])
            nc.sync.dma_start(mm_sb[:], maskm[:, :])
            cw_sb = consts.tile([128, MT_DI * KC], F32)
            nc.sync.dma_start(
                _view3(cw_sb, MT_DI, KC),
                bass.AP(conv_w, 0, [[KC, 128], [128 * KC, MT_DI], [1, KC]]))
            cb_sb = consts.tile([128, MT_DI], F32)
            nc.sync.dma_start(
                cb_sb[:], bass.AP(conv_b, 0, [[1, 128], [128, MT_DI]]))
            eps_sb = consts.tile([128, 1], F32)
            nc.vector.memset(eps_sb[:], EPS)
            ones_sb = consts.tile([128, 1], BF16)
            nc.vector.memset(ones_sb[:], 1.0)
            ones_rowf = consts.tile([1, 128], F32)
            nc.vector.memset(ones_rowf[:], 1.0)
            ones_rowb = consts.tile([1, 128], BF16)
            nc.vector.memset(ones_rowb[:], ASC if G4_FP8 else 1.0)

            x2b = [x2b_pool.tile([128, cext], BF16, name=f"x2b{i}")
                   for i in range(MT_D)]
            x28 = (x28_pool.tile([128, KT * cext], FP8, name="x28")
                   if G3_FP8 else None)
            # aos: fp8 k-major [128, KT*tmain] or bf16 per-hp tiles
            if G4_FP8:
                aos8 = aos_pool.tile([128, KT * tmain], FP8, name="aos8")
            else:
                aosb = [aos_pool.tile([128, tmain], BF16, name=f"ao{i}")
                        for i in range(MT_D)]

            with tc.tile_pool(name="xT", bufs=KT) as xT_pool, \
                 tc.tile_pool(name="zT", bufs=1) as zT_pool, \
                 tc.tile_pool(name="h", bufs=MT_DI) as h_pool:
                xts = []
                for k in range(KT):
                    xt = xT_pool.tile([128, xext], BF16)
                    nc.sync.dma_start(xt[:], xT[k * 128:(k + 1) * 128, :])
                    xts.append(xt)

                # ---- rmsnorm scale r = 1/sqrt(mean(x^2)+eps) ----
                with tc.tile_pool(name="ss_ps", bufs=1, space="PSUM") as ssp, \
                     tc.tile_pool(name="rbp", bufs=2, space="PSUM") as rbp, \
                     tc.tile_pool(name="sq", bufs=1) as sq_pool, \
                     tc.tile_pool(name="tr", bufs=2) as tr_pool:
                    ss_ps = ssp.tile([1, xext], F32)
                    sqs = []
                    for k in range(KT):
                        sq = sq_pool.tile([128, xext], BF16, name=f"sq{k}")
                        nc.vector.tensor_mul(sq[:], xts[k][:], xts[k][:])
                        sqs.append(sq)
                    zts = [zT_pool.tile([128, xext], BF16, name=f"z{k}")
                           for k in range(KT)]
                    for (o, n) in _nslices(xext, 512):
                        for k in range(KT):
                            nc.tensor.matmul(
                                ss_ps[:, o:o + n], ones_sb[:],
                                sqs[k][:, o:o + n],
                                start=(k == 0), stop=(k == KT - 1))
                        t_sb = tr_pool.tile([1, 512], F32, tag="ts")
                        nc.scalar.activation(t_sb[:, 0:n], ss_ps[:, o:o + n],
                                             AF.Sqrt, bias=eps_sb[0:1, :],
                                             scale=1.0 / D)
                        r_sb = tr_pool.tile([1, 512], F32, tag="rs")
                        nc.vector.reciprocal(r_sb[:, 0:n], t_sb[:, 0:n])
                        rb_ps = rbp.tile([128, 512], F32)
                        nc.tensor.matmul(
                            rb_ps[:, 0:n], ones_rowf[:], r_sb[:, 0:n],
                            start=True, stop=True)
                        for k in range(KT):
                            nc.vector.tensor_tensor(
                                out=zts[k][:, o:o + n], in0=xts[k][:, o:o + n],
                                in1=rb_ps[:, 0:n], op=OP.mult)

                # ---- up/gate projections + causal dwconv + silu -> h ----
                hs = []
                with tc.tile_pool(name="wA", bufs=3) as wA_pool, \
                     tc.tile_pool(name="upps", bufs=3, space="PSUM") as upp, \
                     tc.tile_pool(name="gaps", bufs=3, space="PSUM") as gap, \
                     tc.tile_pool(name="usb", bufs=2) as u_pool, \
                     tc.tile_pool(name="silp", bufs=2) as sil_pool, \
                     tc.tile_pool(name="convt", bufs=2) as cv_pool:
                    for m in range(MT_DI):
                        wu = wA_pool.tile([128, KT * 128], BF16, tag="wu")
                        nc.sync.dma_start(
                            wu[:], w_up[m * 128:(m + 1) * 128, :])
                        wg = wA_pool.tile([128, KT * 128], BF16, tag="wg")
                        nc.sync.dma_start(
                            wg[:], w_gate[m * 128:(m + 1) * 128, :])

                        u_sb = u_pool.tile([128, xext], BF16)
                        for (o, n) in _nslices(xext, 512):
                            up_b = upp.tile([128, 512], F32)
                            for k in range(KT):
                                nc.tensor.matmul(
                                    up_b[:, 0:n],
                                    wu[:, k * 128:(k + 1) * 128],
                                    zts[k][:, o:o + n],
                                    start=(k == 0), stop=(k == KT - 1))
                            nc.scalar.activation(
                                u_sb[:, o:o + n], up_b[:, 0:n], AF.Copy)
                        sil = sil_pool.tile([128, cext], BF16)
                        for (o, n) in _nslices(cext, 512):
                            ga_b = gap.tile([128, 512], F32)
                            for k in range(KT):
                                nc.tensor.matmul(
                                    ga_b[:, 0:n],
                                    wg[:, k * 128:(k + 1) * 128],
                                    zts[k][:, KC - 1 + o:KC - 1 + o + n],
                                    start=(k == 0), stop=(k == KT - 1))
                            nc.scalar.activation(
                                sil[:, o:o + n], ga_b[:, 0:n], AF.Silu)
                        # dwconv: acc = sum_j u[:, j:j+cext]*cw[:,j] + b
                        acc = cv_pool.tile([128, cext], BF16, tag="acc")
                        nc.vector.tensor_scalar(
                            out=acc[:], in0=u_sb[:, 0:cext],
                            scalar1=cw_sb[:, m * KC:m * KC + 1],
                            scalar2=cb_sb[:, m:m + 1],
                            op0=OP.mult, op1=OP.add)
                        for j in range(1, KC):
                            tmp = cv_pool.tile([128, cext], BF16, tag="tmp")
                            nc.vector.tensor_scalar(
                                out=tmp[:], in0=u_sb[:, j:j + cext],
                                scalar1=cw_sb[:, m * KC + j:m * KC + j + 1],
                                scalar2=None, op0=OP.mult)
                            nc.vector.tensor_add(acc[:], acc[:], tmp[:])
                        h = h_pool.tile([128, cext], BF16)
                        nc.vector.tensor_mul(h[:], sil[:], acc[:])
                        hs.append(h)

                # ---- down projection + residual -> x2 ----
                with tc.tile_pool(name="wD", bufs=2) as wD_pool, \
                     tc.tile_pool(name="dps", bufs=3, space="PSUM") as d_pool:
                    for m in range(MT_D):
                        wd = wD_pool.tile([128, MT_DI * 128], BF16)
                        nc.sync.dma_start(
                            wd[:], w_down[m * 128:(m + 1) * 128, :])
                        for (o, n) in _nslices(cext, 512):
                            d_b = d_pool.tile([128, 512], F32)
                            for k in range(MT_DI):
                                nc.tensor.matmul(
                                    d_b[:, 0:n],
                                    wd[:, k * 128:(k + 1) * 128],
                                    hs[k][:, o:o + n],
                                    start=(k == 0), stop=(k == MT_DI - 1))
                            nc.vector.tensor_tensor(
                                out=x2b[m][:, o:o + n], in0=d_b[:, 0:n],
                                in1=xts[m][:, KC - 1 + o:KC - 1 + o + n],
                                op=OP.add)
                            if G3_FP8:
                                nc.scalar.activation(
                                    out=_v3(x28, cext, m, 1, o, n),
                                    in_=x2b[m][:, o:o + n],
                                    func=AF.Copy, scale=ASC)
                        if m == 0:
                            tap("t_x2b0", x2b[0][:])
                            tap("t_h0", hs[0][:])

            # ---- qkv projections ----
            with tc.tile_pool(name="qT", bufs=MT_D) as q_pool, \
                 tc.tile_pool(name="kT", bufs=MT_D) as k_pool, \
                 tc.tile_pool(name="vtm", bufs=TT) as v_pool:
                kts = []
                qts = []
                with tc.tile_pool(name="wK", bufs=3) as wK_pool, \
                     tc.tile_pool(name="kps", bufs=6, space="PSUM") as kpp:
                    for m in range(MT_D):
                        wk = wK_pool.tile([128, KT * 128], qk_dt, tag="wk")
                        nc.sync.dma_start(wk[:], w_k[m * 128:(m + 1) * 128, :])
                        kt = k_pool.tile([128, cext], BF16)
                        for (o, n) in _nslices(cext, 512):
                            k_b = kpp.tile([128, 512], F32, tag="kb")
                            if G3_FP8:
                                for k in range(0, KT, 2):
                                    nc.tensor.matmul(
                                        k_b[:, 0:n],
                                        _v3(wk, 128, k, 2, 0, 128),
                                        _v3(x28, cext, k, 2, o, n),
                                        start=(k == 0), stop=(k == KT - 2),
                                        perf_mode=DR)
                            else:
                                for k in range(KT):
                                    nc.tensor.matmul(
                                        k_b[:, 0:n],
                                        wk[:, k * 128:(k + 1) * 128],
                                        x2b[k][:, o:o + n],
                                        start=(k == 0), stop=(k == KT - 1))
                            nc.scalar.activation(
                                kt[:, o:o + n], k_b[:, 0:n], AF.Copy,
                                scale=qkv_desc)
                        kts.append(kt)
                        if m == 0:
                            tap("t_kt0", kt[:])
                    for m in range(MT_D):
                        wq = wK_pool.tile([128, KT * 128], qk_dt, tag="wq")
                        nc.sync.dma_start(wq[:], w_q[m * 128:(m + 1) * 128, :])
                        qt = q_pool.tile([128, tmain], BF16)
                        for (o, n) in _nslices(tmain, 512):
                            q_b = kpp.tile([128, 512], F32, tag="kb")
                            if G3_FP8:
                                for k in range(0, KT, 2):
                                    nc.tensor.matmul(
                                        q_b[:, 0:n],
                                        _v3(wq, 128, k, 2, 0, 128),
                                        _v3(x28, cext, k, 2, halo + o, n),
                                        start=(k == 0), stop=(k == KT - 2),
                                        perf_mode=DR)
                            else:
                                for k in range(KT):
                                    nc.tensor.matmul(
                                        q_b[:, 0:n],
                                        wq[:, k * 128:(k + 1) * 128],
                                        x2b[k][:, halo + o:halo + o + n],
                                        start=(k == 0), stop=(k == KT - 1))
                            nc.scalar.activation(
                                qt[:, o:o + n], q_b[:, 0:n], AF.Copy,
                                scale=qkv_desc)
                        qts.append(qt)
                        if m == 0:
                            tap("t_qt0", qt[:])

                vts = []
                with tc.tile_pool(name="wV", bufs=1) as wV_pool, \
                     tc.tile_pool(name="vps", bufs=3, space="PSUM") as vpp:
                    wv = wV_pool.tile([128, KT * 1024], qk_dt)
                    nc.sync.dma_start(wv[:], w_v[:, :])
                    for tt in range(TT):
                        v_ps = vpp.tile([128, 1024], F32)
                        if G3_FP8:
                            for (o, n) in _nslices(1024, 512):
                                for k in range(0, KT, 2):
                                    nc.tensor.matmul(
                                        v_ps[:, o:o + n],
                                        _v3(x28, cext, k, 2, tt * 128, 128),
                                        _v3(wv, 1024, k, 2, o, n),
                                        start=(k == 0), stop=(k == KT - 2),
                                        perf_mode=DR)
                        else:
                            for (o, n) in _nslices(1024, 512):
                                for k in range(KT):
                                    nc.tensor.matmul(
                                        v_ps[:, o:o + n],
                                        x2b[k][:, tt * 128:(tt + 1) * 128],
                                        wv[:, k * 1024 + o:k * 1024 + o + n],
                                        start=(k == 0), stop=(k == KT - 1))
                        vt = v_pool.tile([128, H * (HD + 1)], BF16)
                        nc.vector.memset(
                            bass.AP(vt.tensor, vt[:].offset + HD,
                                    [vt[:].ap[0], [HD + 1, H], [1, 1]]), 1.0)
                        for (o, n) in _nslices(1024, 512):
                            nh = n // HD
                            dst = bass.AP(vt.tensor,
                                          vt[:].offset + (o // HD) * (HD + 1),
                                          [vt[:].ap[0], [HD + 1, nh], [1, HD]])
                            src = bass.AP(v_ps.tensor, v_ps[:].offset + o,
                                          [v_ps[:].ap[0], [HD, nh], [1, HD]])
                            nc.scalar.activation(dst, src, AF.Copy,
                                                 scale=qkv_desc)
                        vts.append(vt)
                        if tt == 0:
                            tap("t_vt0", vt[:])

                # ---- sliding-window attention ----
                # compact scores layout: cols [0:256]=s1(q 0:256),
                # [256:512]=s2(q 0:256), [512:640]=s0(q 0:128),
                # [640:768]=s3(q 128:256)
                with tc.tile_pool(name="sps", bufs=2, space="PSUM") as s_pool, \
                     tc.tile_pool(name="ops", bufs=2, space="PSUM") as o_pool, \
                     tc.tile_pool(name="rbps", bufs=2, space="PSUM") as rb_pool, \
                     tc.tile_pool(name="adp", bufs=3) as ad_pool, \
                     tc.tile_pool(name="esb", bufs=3) as e_pool, \
                     tc.tile_pool(name="ocp", bufs=36) as oc_pool, \
                     tc.tile_pool(name="denp", bufs=2) as den_pool, \
                     tc.tile_pool(name="rcpp", bufs=2) as rcp_pool:
                    pend = []

                    def scores_stage(c, hh):
                        msk = m0_sb if c == 0 else mm_sb
                        hp, x = hh >> 1, hh & 1
                        kt, qt = kts[hp], qts[hp]
                        po = x * 64
                        s_ps = s_pool.tile([128, 768], F32, tag="spt")
                        qb = c * WIN
                        for (dst, ks, qo, qn) in (
                                (0, 1, 0, 256), (256, 2, 0, 256),
                                (512, 0, 0, 128), (640, 3, 128, 128)):
                            nc.tensor.matmul(
                                s_ps[:, dst:dst + qn],
                                kt[po:po + 64,
                                   qb + ks * 128:qb + (ks + 1) * 128],
                                qt[po:po + 64, qb + qo:qb + qo + qn],
                                start=True, stop=True,
                                tile_position=(po, 0))
                        ad = ad_pool.tile([128, 768], BF16)
                        nc.vector.tensor_tensor(
                            out=ad[:], in0=s_ps[:], in1=msk[:], op=OP.add)
                        ee = e_pool.tile([128, 768], BF16)
                        nc.scalar.activation(ee[:], ad[:], AF.Exp)
                        return ee

                    def av_stage(c, hh, ee, den_sb, ocs):
                        o_ps = o_pool.tile([65, 256], F32)
                        vsl = [vts[c * 2 + s][
                            :, hh * (HD + 1):(hh + 1) * (HD + 1)]
                            for s in range(4)]
                        nc.tensor.matmul(o_ps[:, 0:256], vsl[1],
                                         ee[:, 0:256],
                                         start=True, stop=False)
                        nc.tensor.matmul(o_ps[:, 0:256], vsl[2],
                                         ee[:, 256:512],
                                         start=False, stop=False,
                                         skip_group_check=True)
                        nc.tensor.matmul(o_ps[:, 0:128], vsl[0],
                                         ee[:, 512:640],
                                         start=False, stop=False,
                                         skip_group_check=True)
                        nc.tensor.matmul(o_ps[:, 128:256], vsl[3],
                                         ee[:, 640:768],
                                         start=False, stop=True,
                                         skip_group_check=True)
                        oc = oc_pool.tile([65, 256], BF16)
                        nc.scalar.activation(oc[:], o_ps[:, :], AF.Copy)
                        ocs.append(oc)
                        nc.sync.dma_start(den_sb[hh:hh + 1, :],
                                          oc[64:65, :])

                    def division_start(c, den_sb):
                        rcp = rcp_pool.tile([16, 256], BF16, tag="rc")
                        with nc.allow_low_precision(
                                reason="softmax denom recip in bf16"):
                            nc.vector.reciprocal(rcp[:], den_sb[:])
                        rowb = rcp_pool.tile([1, 16 * 256], BF16, tag="rw")
                        nc.sync.dma_start(
                            bass.AP(rowb.tensor, rowb[:].offset,
                                    [[rowb[:].ap[0][0], 1], [256, 16],
                                     [1, 256]]),
                            rcp[:])
                        return rowb

                    def division_head(c, hh, rowb, ocs):
                        hp, x = hh >> 1, hh & 1
                        rb2 = rb_pool.tile([64, 256], F32)
                        nc.tensor.matmul(
                            rb2[:], ones_rowb[:, 0:64],
                            rowb[0:1, hh * 256:(hh + 1) * 256],
                            start=True, stop=True)
                        if G4_FP8:
                            dst = aos8[x * 64:(x + 1) * 64,
                                       hp * tmain + c * WIN:
                                       hp * tmain + (c + 1) * WIN]
                        else:
                            dst = aosb[hp][x * 64:(x + 1) * 64,
                                           c * WIN:(c + 1) * WIN]
                        nc.vector.tensor_tensor(
                            out=dst, in0=ocs[hh][0:64, :],
                            in1=rb2[:], op=OP.mult)

                    dens, ocss = {}, {}
                    divq = []
                    divheads = []

                    def pop_av():
                        pc, ph, pee = pend.pop(0)
                        av_stage(pc, ph, pee, dens[pc], ocss[pc])
                        if ph == H - 1:
                            divq.append(pc)

                    def pump(ndiv):
                        if divq and not divheads:
                            dc = divq.pop(0)
                            rw = division_start(dc, dens[dc])
                            divheads.extend(
                                (dc, hh, rw) for hh in range(H))
                        for _ in range(ndiv):
                            if divheads:
                                dc, dh, rw = divheads.pop(0)
                                division_head(dc, dh, rw, ocss[dc])

                    for c in range(nchunk):
                        dens[c] = den_pool.tile([16, 256], BF16,
                                                name=f"den{c & 1}")
                        ocss[c] = []
                        for hh in range(H):
                            pend.append((c, hh, scores_stage(c, hh)))
                            if len(pend) > 1:
                                pop_av()
                            if hh >= 4:
                                pump(2)
                    while pend:
                        pop_av()
                    while divq or divheads:
                        pump(4)

                # ---- output projection + residual ----
                if G4_FP8:
                    tap("t_aos", aos8[:])
                with tc.tile_pool(name="wO", bufs=2) as wO_pool, \
                     tc.tile_pool(name="wops", bufs=3, space="PSUM") as wop, \
                     tc.tile_pool(name="osb", bufs=3) as out_pool:
                    for m in range(MT_D):
                        wo = wO_pool.tile([128, KT * 128], o_dt)
                        nc.sync.dma_start(wo[:], w_o[m * 128:(m + 1) * 128, :])
                        for (o, n) in _nslices(tmain, 512):
                            wo_b = wop.tile([128, 512], F32)
                            if G4_FP8:
                                for k in range(0, KT, 2):
                                    nc.tensor.matmul(
                                        wo_b[:, 0:n],
                                        _v3(wo, 128, k, 2, 0, 128),
                                        _v3(aos8, tmain, k, 2, o, n),
                                        start=(k == 0), stop=(k == KT - 2),
                                        perf_mode=DR)
                            else:
                                for k in range(KT):
                                    nc.tensor.matmul(
                                        wo_b[:, 0:n],
                                        wo[:, k * 128:(k + 1) * 128],
                                        aosb[k][:, o:o + n],
                                        start=(k == 0), stop=(k == KT - 1))
                            ot = out_pool.tile([128, 512], F32)
                            nc.vector.scalar_tensor_tensor(
                                out=ot[:, 0:n], in0=wo_b[:, 0:n],
                                scalar=o_desc,
                                in1=x2b[m][:, halo + o:halo + o + n],
                                op0=OP.mult, op1=OP.add)
                            nc.sync.dma_start(
                                outT[m * 128:(m + 1) * 128, o:o + n],
                                ot[:, 0:n])
    _split_excess_waits(nc, wsem)
    return nc


def make_masks():
    """Compact [128, 768] masks: cols [0:256]=s1, [256:512]=s2,
    [512:640]=s0 (q 0:128), [640:768]=s3 (q 128:256)."""
    i = np.arange(WIN)[None, :]          # query
    p = np.arange(2 * WIN)[:, None]      # key
    band = (p > i) & (p <= i + WIN)
    mm_full = np.where(band, SHIFT, NEG).astype(np.float32)   # [512, 256]
    m0_full = np.where(band & (p >= WIN), SHIFT, NEG).astype(np.float32)

    def compact(mfull):
        out = np.full((128, 768), NEG, np.float32)
        out[:, 0:256] = mfull[128:256, :]          # s1
        out[:, 256:512] = mfull[256:384, :]        # s2
        out[:, 512:640] = mfull[0:128, 0:128]      # s0, q<128
        out[:, 640:768] = mfull[384:512, 128:256]  # s3, q>=128
        return out
    return compact(m0_full), compact(mm_full)


def _lhsT_img(W, scale=1.0):
    """[Din, Dout] weight -> lhsT image [Dout, KTin*128] with
    img[mo*128+f, k*128+p] = W[k*128+p, mo*128+f]*scale."""
    Din, Dout = W.shape
    KTin = Din // 128
    MT = Dout // 128
    img = (W * scale).reshape(KTin, 128, MT, 128).transpose(2, 1, 0, 3)
    return np.ascontiguousarray(img.reshape(Dout, KTin * 128))


def make_core_inputs(x, norm_w, w_up, w_gate, w_down, conv_w, conv_b,
                     w_qkv, w_o, tmain, s_total):
    halo = WIN
    xext = tmain + halo + (KC - 1)
    bf = ml_dtypes.bfloat16
    f8 = mybir.dt.np(FP8)
    qk_np = f8 if G3_FP8 else bf
    o_np = f8 if G4_FP8 else bf
    qk_sc = WSC if G3_FP8 else 1.0
    o_sc = WSC if G4_FP8 else 1.0

    wu_img = _lhsT_img(norm_w[:, None] * w_up).astype(bf)
    wg_img = _lhsT_img(norm_w[:, None] * w_gate).astype(bf)
    wd_img = _lhsT_img(w_down).astype(bf)
    wq = w_qkv[:, 0:D] * np.float32(1.0 / np.sqrt(HD))
    wq_img = _lhsT_img(wq, qk_sc).astype(qk_np)
    wk_img = _lhsT_img(w_qkv[:, D:2 * D], qk_sc).astype(qk_np)
    # v weights as rhs image: [128, KT*1024], [p, k*1024+f]
    wv = (w_qkv[:, 2 * D:3 * D] * qk_sc).reshape(D // 128, 128, D)
    wv_img = np.ascontiguousarray(
        wv.transpose(1, 0, 2).reshape(128, (D // 128) * D)).astype(qk_np)
    wo_img = _lhsT_img(w_o, o_sc).astype(o_np)
    m0, mm = make_masks()
    m0 = m0.astype(bf)
    mm = mm.astype(bf)
    cores_per_b = s_total // tmain
    in_maps = []
    for c in range(NCORES):
        b, blk = divmod(c, cores_per_b)
        t0 = blk * tmain
        lo = t0 - halo - (KC - 1)
        xe = x[b, max(0, lo):t0 + tmain]
        if lo < 0:
            xe = np.concatenate(
                [np.zeros((-lo, D), np.float32), xe], axis=0)
        xT = np.ascontiguousarray(xe.T).astype(bf)
        assert xT.shape == (D, xext)
        in_maps.append({
            "xT": xT,
            "w_up": wu_img, "w_gate": wg_img, "w_down": wd_img,
            "w_q": wq_img, "w_k": wk_img, "w_v": wv_img, "w_o": wo_img,
            "conv_w": conv_w.astype(np.float32),
            "conv_b": conv_b.astype(np.float32),
            "mask0": m0 if blk == 0 else mm,
            "maskm": mm,
        })
    return in_maps


_CACHED = {}


def kernel(x, norm_w, w_up, w_gate, w_down, conv_w, conv_b, w_qkv, w_o):
    from concourse.bass_utils import run_bass_kernel_spmd
    tmain = (B * S) // NCORES
    if "nc" not in _CACHED:
        _CACHED["nc"] = build_program(tmain)
    nc = _CACHED["nc"]
    in_maps = make_core_inputs(
        np.asarray(x, np.float32), np.asarray(norm_w, np.float32),
        np.asarray(w_up, np.float32), np.asarray(w_gate, np.float32),
        np.asarray(w_down, np.float32), np.asarray(conv_w, np.float32),
        np.asarray(conv_b, np.float32), np.asarray(w_qkv, np.float32),
        np.asarray(w_o, np.float32), tmain, S)
    res = run_bass_kernel_spmd(nc, in_maps, core_ids=list(range(NCORES)))
    out = np.empty((B, S, D), np.float32)
    cores_per_b = S // tmain
    for c in range(NCORES):
        b, blk = divmod(c, cores_per_b)
        out[b, blk * tmain:(blk + 1) * tmain] = res.results[c]["outT"].T
    return out
